# revision 28
# baseline (speedup 1.0000x reference)
"""DeepFM (nn_DeepFM_77120432766994) Trainium2 Bass kernel, v2.

Strategy: data-parallel over batch across 8 NeuronCores; per core 2048
batch rows in 4 tiles of 512.

v2 redesign of the gather plan (v1 spent ~75% of its time on Pool-engine
SWDGE descriptor generation: 48 DMA instructions/core at ~1-4us each):

  - 6 "gathered" fields (0,1,2,3,5,6; vocab 1e6..500): ONE multi-offset
    indirect DMA per tile ([128, 24] int32 offset AP = 3072 descriptors
    in a single instruction, 994ns fixed + 0.34ns/desc generation).
    Rows are 33 f32 (132B, vs v1's 256B-padded gather rows) and land
    directly in the packed [p, subtile, field*33+e] staging layout the
    PE transposes consume - no DVE repack at all.
  - 4 "tiny" fields (4,7,8,9; vocab 102,50,112,107 <= 128): no DMA.
    One-hot matmul gather: PE replicates the index row across
    partitions (ones-matmul), DVE builds a bf16 one-hot mask via a
    fused subtract+is_equal tensor_scalar, and 3 bf16 matmul passes per
    field against a precomputed product table
    U = [T@W0_f.T | T | bias | -0.5*sum(T^2)]
    accumulate the field's MLP-layer-0 contribution directly into the
    h1 PSUM banks and its FM contributions (field-sum rows, bias sum,
    -0.5*sum e^2) into the shared S bank. Tiny fields therefore never
    materialize embeddings at all.

Pool instructions per tile: 1 (vs v1's 12). Per core: 4 (vs 48).

Compute: as v1 - feature-major X chunks (128 + 86 rows incl. dense),
masked FM matmuls, 0.5*||s||^2 via ACT square + matmul, MLP chain with
fused bias+ReLU, everything funnelled into one PSUM bank that holds the
S rows (0:33), tiny-q row (33), fm accumulator row (35) and h3 block
(rows 64:128); sigmoid on ACT; y out via DMA.
"""

import os
import sys

import numpy as np

for _p in ("/opt/trn_rl_repo", "/root/.axon_site/_ro/trn_rl_repo"):
    if os.path.isdir(_p) and _p not in sys.path:
        sys.path.insert(0, _p)

import ml_dtypes

import concourse.bass as bass
import concourse.bacc as bacc
import concourse.mybir as mybir
import concourse.tile as tile
from concourse.bass_utils import run_bass_kernel_spmd

# ---------------------------------------------------------------- constants
FIELD_DIMS = [1000000, 100000, 1008, 1004, 102, 1000, 500, 50, 112, 107]
NF = 10
GATH_F = [0, 1, 2, 3, 5, 6]   # feature order: f0,f1 indirect; f2,f3,f5,f6 gather
BIG_F = [0, 1]                # per-partition indirect DMA (global int32 rows)
SM_F = [2, 3, 5, 6]           # dma_gather from 256B-row subtable (int16 local)
OH_F = [4, 7, 8, 9]           # vocab <= 128: one-hot matmul, no DMA
NG = len(GATH_F)              # 6
NBIG = len(BIG_F)             # 2
NSM = len(SM_F)               # 4
NOH = len(OH_F)               # 4
VSM = int(np.sum([FIELD_DIMS[f] for f in SM_F]))              # 3512
SM_OFF = np.concatenate([[0], np.cumsum(
    [FIELD_DIMS[f] for f in SM_F])[:-1]]).astype(np.int64)
GW = 64                       # gather row width (256B: 33 used + pad)
NGI = 1024                    # idx per gather (SWDGE ring capacity)
NGATH = NSM * 512 // NGI      # 2 gathers per tile (BT=512)
I16 = mybir.dt.int16
B = 16384
EMB = 32
DENSE = 16
MLP = [256, 128, 64]
BN_EPS = 1e-5
V = int(np.sum(FIELD_DIMS))
OFFSETS = np.concatenate([[0], np.cumsum(FIELD_DIMS)[:-1]]).astype(np.int64)

N_CORES = 8
B_LOC = B // N_CORES          # 2048
NT = 4                        # tiles per core
BT = 512                      # batch per tile
NS = 4                        # subtiles (of 128) per tile
P = 128

FW = 33                       # per-field feature width (32 emb + bias)
GFEAT = NG * FW               # 198 gathered feature rows
KC0 = P                       # chunk 0 rows (gathered features 0:128)
GC1 = GFEAT - P               # 70 gathered rows in chunk 1
DROW = GC1                    # dense rows start here in chunk 1
KC1 = GC1 + DENSE             # 86 chunk-1 contraction size

UW = 2 * P + 2 * EMB          # 320 tiny product-table cols per field
U_Z0 = 0                      # h1a neurons 0:128
U_Z1 = P                      # h1b neurons 128:256
U_BQ = 2 * P                  # col 256: bias - 0.5*sum(T^2) -> fm row 0
U_S = 2 * P + EMB             # s-table cols 288:320 -> S rows 32:64

F32 = mybir.dt.float32
F32R = mybir.dt.float32r
BF16 = mybir.dt.bfloat16
I32 = mybir.dt.int32

MMDT = F32R

# wall (f32r weight blob) column offsets
W0C0_O = 0                    # [128, 256]
W0C1_O = 256                  # [86, 256]
W1_O = 512                    # 2 chunks x 128
W2_O = 768                    # 64
WO_O = 832                    # 1 (rows 0:64)
FMS0_O = 833                  # 64 (col 0 = bias->fm row 0, 32:64 = s rows)
FMS1_O = 897                  # 64
FMQ0_O = 961                  # 1
FMQ1_O = 962                  # 1
HV_O = 963                    # 1 (rows 32:64, value 0.5)
ONES_O = 964                  # 128 (row 0 only: broadcast ones)
WALL_W = 1092
# tiny (bf16 blob): 4 fields x UW product tables + bf16 identity
TID_O = NOH * UW              # 1160
TINY_W = TID_O + P            # 1288
# aux (f32 blob) column offsets
ID_O = 0                      # identity 128
B1_O = 128                    # 2
B2_O = 130                    # 1
B3_O = 131                    # 1 (rows 0:64 = b2f)
SC_O = 132                    # 1 (row 0: bo + fm_bias)
IOTA_O = 133                  # 1 (row p: float(p))
AUX_W = 134

# S/fm/h3 shared PSUM bank rows. Row 0 is THE scalar accumulator: every
# per-sample scalar term (field-bias sums, tiny bias-0.5*sum e^2, gathered
# -0.5*sum e^2, 0.5*||s||^2, Wo@h3 deep output) lands there via matmul
# accumulation, so the presigmoid is read straight out of row 0. s rows
# sit at 32:64: matmul dst/lhsT bases and PSUM reads must be 32-aligned,
# and ACT cannot shift partitions, so s/S2/HV all stay at base 32.
SR_FM = 0                     # scalar accumulator row
SR_S = EMB                    # s rows 32:64


# ---------------------------------------------------------------- device code
def _build_nc(reps=1):
    nc = bacc.Bacc("TRN2", target_bir_lowering=False, debug=False)

    tab = nc.dram_tensor("tab", [V, FW], F32, kind="ExternalInput").ap()
    tabs = nc.dram_tensor("tabs", [VSM, GW], F32, kind="ExternalInput").ap()
    gidx = nc.dram_tensor("gidx", [NT, P, NS * NBIG], I32,
                          kind="ExternalInput").ap()
    sidx = nc.dram_tensor("sidx", [NT, P, NGATH * (NGI // 16)], I16,
                          kind="ExternalInput").ap()
    ohidx = nc.dram_tensor("ohidx", [NT, 1, NOH * BT], MMDT,
                           kind="ExternalInput").ap()
    denset = nc.dram_tensor("denset", [NT, DENSE, BT], MMDT,
                            kind="ExternalInput").ap()
    walld = nc.dram_tensor("walld", [P, WALL_W], MMDT, kind="ExternalInput").ap()
    tinyd = nc.dram_tensor("tinyd", [P, TINY_W], BF16, kind="ExternalInput").ap()
    auxd = nc.dram_tensor("auxd", [P, AUX_W], F32, kind="ExternalInput").ap()
    y = nc.dram_tensor("y", [NT, BT], F32, kind="ExternalOutput").ap()

    from contextlib import ExitStack
    with tile.TileContext(nc) as tc, ExitStack() as ctx:
        wp = ctx.enter_context(tc.tile_pool(name="weights", bufs=1))

        wall = wp.tile([P, WALL_W], MMDT, tag="wall")
        nc.sync.dma_start(out=wall[:], in_=walld[:])
        tiny = wp.tile([P, TINY_W], BF16, tag="tiny")
        nc.sync.dma_start(out=tiny[:], in_=tinyd[:])
        aux = wp.tile([P, AUX_W], F32, tag="aux")
        nc.sync.dma_start(out=aux[:], in_=auxd[:])
        dummy = wp.tile([1, 2], F32, tag="dummy")

        ident = aux[:, ID_O:ID_O + P]
        iota = aux[:, IOTA_O:IOTA_O + 1]

        # SBUF pools
        ip = ctx.enter_context(tc.tile_pool(name="idx", bufs=2))
        sip = ctx.enter_context(tc.tile_pool(name="sidxp", bufs=2))
        gsp = ctx.enter_context(tc.tile_pool(name="gsmall", bufs=2))
        gfp = ctx.enter_context(tc.tile_pool(name="gf01", bufs=2))
        oip = ctx.enter_context(tc.tile_pool(name="ohidx", bufs=2))
        gp = ctx.enter_context(tc.tile_pool(name="gstage", bufs=2))
        mkp = ctx.enter_context(tc.tile_pool(name="masks", bufs=2 * NOH))
        xp = ctx.enter_context(tc.tile_pool(name="xchunks", bufs=4))
        xqp = ctx.enter_context(tc.tile_pool(name="xsq", bufs=4))
        s2p = ctx.enter_context(tc.tile_pool(name="s2", bufs=4))
        hp = ctx.enter_context(tc.tile_pool(name="acts", bufs=8))
        yp = ctx.enter_context(tc.tile_pool(name="yout", bufs=2))

        # PSUM pools: 8 banks total. ps_bc doubles as the h2 bank (the 4
        # bcast tiles are dead after their compares, well before MLP1);
        # ps_sfm is double-buffered so tile t+1's S-chain doesn't wait on
        # tile t's sigmoid read.
        ps_x0 = ctx.enter_context(tc.tile_pool(name="ps_x0", bufs=1, space="PSUM"))
        ps_x1 = ctx.enter_context(tc.tile_pool(name="ps_x1", bufs=1, space="PSUM"))
        ps_bc = ctx.enter_context(tc.tile_pool(name="ps_bc", bufs=2, space="PSUM"))
        ps_h1a = ctx.enter_context(tc.tile_pool(name="ps_h1a", bufs=1, space="PSUM"))
        ps_h1b = ctx.enter_context(tc.tile_pool(name="ps_h1b", bufs=1, space="PSUM"))
        ps_sfm = ctx.enter_context(tc.tile_pool(name="ps_sfm", bufs=2, space="PSUM"))

        def w0(c, o):
            base = W0C0_O if c == 0 else W0C1_O
            kc = KC0 if c == 0 else KC1
            return wall[0:kc, base + o * P:base + (o + 1) * P]

        def w1(k):
            return wall[:, W1_O + k * MLP[1]:W1_O + (k + 1) * MLP[1]]

        def fmw_s(c):
            base, kc = (FMS0_O, KC0) if c == 0 else (FMS1_O, KC1)
            return wall[0:kc, base:base + SR_S + EMB]

        def fmw_q(c):
            base, kc = (FMQ0_O, KC0) if c == 0 else (FMQ1_O, GC1)
            return wall[0:kc, base:base + 1]

        # Absorber instructions: make PE/ACT/DVE observe the weight DMA
        # semaphores once, before any real consumer.
        xps_a = ps_x0.tile([P, BT], F32, tag="x0")
        nc.tensor.transpose(out=xps_a[:, 0:P], in_=ident, identity=ident)
        nc.tensor.transpose(out=xps_a[:, P:2 * P], in_=wall[:, 0:P].bitcast(F32),
                            identity=ident)
        nc.tensor.transpose(out=xps_a[:, 2 * P:3 * P].bitcast(BF16)[:, 0:P],
                            in_=tiny[:, 0:P],
                            identity=tiny[:, TID_O:TID_O + P])
        nc.scalar.copy(dummy[:, 0:1], aux[0:1, B1_O:B1_O + 1])
        nc.vector.tensor_copy(dummy[:, 1:2], aux[0:1, ID_O:ID_O + 1])

        import contextlib
        loop_cm = tc.For_i(0, reps, 1) if reps > 1 else contextlib.nullcontext()
        with loop_cm:
          for t in range(NT):
            # ---- small fields f2,f3,f5,f6: 2 gathers of 1024 rows
            si = sip.tile([P, NGATH * (NGI // 16)], I16, tag="si")
            nc.sync.dma_start(out=si[:], in_=sidx[t])
            Gs = gsp.tile([P, NS * NSM * GW], F32, tag="Gs")
            Gsv = Gs[:].rearrange("p (c e) -> p c e", c=NS * NSM)
            W16 = NGI // 16
            for g in range(NGATH):
                nc.gpsimd.dma_gather(
                    out_ap=Gsv[:, g * (NGI // P):(g + 1) * (NGI // P), :],
                    in_ap=tabs,
                    idxs_ap=si[:, g * W16:(g + 1) * W16],
                    num_idxs=NGI,
                    num_idxs_reg=NGI,
                    elem_size=GW,
                    transpose=False,
                )

            # ---- big fields f0,f1: per-partition indirect DMA (8 insts)
            gi = ip.tile([P, NS * NBIG], I32, tag="gi")
            nc.sync.dma_start(out=gi[:], in_=gidx[t])
            Gf = gfp.tile([P, NS * NBIG * FW], F32, tag="Gf")
            Gf3 = Gf[:].rearrange("p (s f) -> p s f", s=NS)
            for s in range(NS):
                for f in range(NBIG):
                    j = s * NBIG + f
                    nc.gpsimd.indirect_dma_start(
                        out=Gf3[:, s, f * FW:(f + 1) * FW],
                        out_offset=None,
                        in_=tab,
                        in_offset=bass.IndirectOffsetOnAxis(
                            ap=gi[:, j:j + 1], axis=0),
                    )

            # ---- DVE pack -> G6 [128, NS, 198] = [f0 f1 | f2 f3 f5 f6]
            G6 = gp.tile([P, NS * NG * FW], F32, tag="G6")
            G6v = G6[:].rearrange("p (s c) -> p s c", s=NS)
            G4 = G6[:].rearrange("p (s fi e) -> p s fi e", s=NS, fi=NG)
            Gf4 = Gf[:].rearrange("p (s fi e) -> p s fi e", s=NS, fi=NBIG)
            nc.vector.tensor_copy(G4[:, :, 0:NBIG, :], Gf4[:])
            Gs4 = Gs[:].rearrange("p (s fi e) -> p s fi e", s=NS, fi=NSM)
            nc.vector.tensor_copy(G4[:, :, NBIG:NG, 0:FW], Gs4[:, :, :, 0:FW])

            oi = oip.tile([1, NOH * BT], MMDT, tag="oi")
            nc.sync.dma_start(out=oi[:], in_=ohidx[t])

            # ---- transpose gathered features to feature-major chunks
            xc0 = ps_x0.tile([P, BT], F32, tag="x0")
            xc1 = ps_x1.tile([P, BT], F32, tag="x1")
            for s in range(NS):
                nc.tensor.transpose(out=xc0[:, s * P:(s + 1) * P],
                                    in_=G6v[:, s, 0:P], identity=ident)
                nc.tensor.transpose(out=xc1[0:GC1, s * P:(s + 1) * P],
                                    in_=G6v[:, s, P:GFEAT], identity=ident)
            X0 = xp.tile([P, BT], MMDT, tag="X")
            nc.vector.tensor_copy(X0[:], xc0[:])
            X1 = xp.tile([P, BT], MMDT, tag="X")
            nc.vector.tensor_copy(X1[0:GC1, :], xc1[0:GC1, :])
            nc.sync.dma_start(out=X1[DROW:DROW + DENSE, :], in_=denset[t])

            # ---- tiny fields: bcast idx, build bf16 one-hot masks
            masks = []
            for o in range(NOH):
                bc = ps_bc.tile([P, BT], F32, tag="bc")
                nc.tensor.matmul(bc[:], lhsT=wall[0:1, ONES_O:ONES_O + P],
                                 rhs=oi[0:1, o * BT:(o + 1) * BT],
                                 start=True, stop=True)
                mk = mkp.tile([P, BT], BF16, tag="mk")
                nc.vector.tensor_scalar(
                    out=mk[:], in0=bc[:], scalar1=iota, scalar2=0.0,
                    op0=mybir.AluOpType.subtract, op1=mybir.AluOpType.is_equal)
                masks.append(mk)

            # ---- shared S/fm/h3 bank + h1 banks
            sp = ps_sfm.tile([P, BT], F32, tag="sfm")
            h1a = ps_h1a.tile([P, BT], F32, tag="h1a")
            h1b = ps_h1b.tile([P, BT], F32, tag="h1b")

            # tiny folded passes: h1 + [s | bias-0.5*sum e^2] contributions
            for o in range(NOH):
                U = tiny[:, o * UW:(o + 1) * UW]
                nc.tensor.matmul(h1a[:], lhsT=U[:, U_Z0:U_Z0 + P],
                                 rhs=masks[o][:], start=(o == 0), stop=False)
                nc.tensor.matmul(h1b[:], lhsT=U[:, U_Z1:U_Z1 + P],
                                 rhs=masks[o][:], start=(o == 0), stop=False)
                nc.tensor.matmul(sp[0:SR_S + EMB, :],
                                 lhsT=U[:, U_BQ:U_BQ + SR_S + EMB],
                                 rhs=masks[o][:], start=(o == 0), stop=False)

            # gathered-field FM: bias -> row 0, s -> rows 32:64
            nc.tensor.matmul(sp[0:SR_S + EMB, :], lhsT=fmw_s(0), rhs=X0[:],
                             start=False, stop=False)
            nc.tensor.matmul(sp[0:SR_S + EMB, :], lhsT=fmw_s(1),
                             rhs=X1[0:KC1, :], start=False, stop=True)

            # gathered-field -0.5*sum e^2 into fm row (squares read the
            # transpose PSUM banks directly; dense rows are q-masked out)
            Xq0 = xqp.tile([P, BT], MMDT, tag="Xq")
            nc.scalar.square(Xq0[:], xc0[:])
            Xq1 = xqp.tile([P, BT], MMDT, tag="Xq")
            nc.scalar.square(Xq1[0:GC1, :], xc1[0:GC1, :])
            nc.tensor.matmul(sp[SR_FM:SR_FM + 1, :], lhsT=fmw_q(0), rhs=Xq0[:],
                             start=False, stop=False)
            nc.tensor.matmul(sp[SR_FM:SR_FM + 1, :], lhsT=fmw_q(1),
                             rhs=Xq1[0:GC1, :], start=False, stop=False)

            # 0.5*||s||^2 into fm row (everything stays at base 32)
            S2 = s2p.tile([SR_S + EMB, BT], MMDT, tag="S2")
            nc.scalar.square(S2[SR_S:SR_S + EMB, :], sp[SR_S:SR_S + EMB, :])
            nc.tensor.matmul(sp[SR_FM:SR_FM + 1, :],
                             lhsT=wall[SR_S:SR_S + EMB, HV_O:HV_O + 1],
                             rhs=S2[SR_S:SR_S + EMB, :],
                             start=False, stop=False)

            # ---- MLP layer 0 (gathered part; tiny already accumulated)
            nc.tensor.matmul(h1a[:], lhsT=w0(0, 0), rhs=X0[:],
                             start=False, stop=False)
            nc.tensor.matmul(h1a[:], lhsT=w0(1, 0), rhs=X1[0:KC1, :],
                             start=False, stop=True)
            nc.tensor.matmul(h1b[:], lhsT=w0(0, 1), rhs=X0[:],
                             start=False, stop=False)
            nc.tensor.matmul(h1b[:], lhsT=w0(1, 1), rhs=X1[0:KC1, :],
                             start=False, stop=True)
            h1sa = hp.tile([P, BT], MMDT, tag="h1s")
            nc.scalar.activation(h1sa[:], h1a[:],
                                 mybir.ActivationFunctionType.Relu,
                                 bias=aux[:, B1_O:B1_O + 1])
            h1sb = hp.tile([P, BT], MMDT, tag="h1s")
            nc.scalar.activation(h1sb[:], h1b[:],
                                 mybir.ActivationFunctionType.Relu,
                                 bias=aux[:, B1_O + 1:B1_O + 2])

            # layer 1
            h2p = ps_bc.tile([P, BT], F32, tag="bc")
            nc.tensor.matmul(h2p[:], lhsT=w1(0), rhs=h1sa[:],
                             start=True, stop=False)
            nc.tensor.matmul(h2p[:], lhsT=w1(1), rhs=h1sb[:],
                             start=False, stop=True)
            h2 = hp.tile([P, BT], MMDT, tag="h2")
            nc.scalar.activation(h2[:], h2p[:],
                                 mybir.ActivationFunctionType.Relu,
                                 bias=aux[:, B2_O:B2_O + 1])

            # layer 2 into the x1 bank (dead after the X1 copy/square; PE
            # reaches the next tile's transposes only after Wo anyway)
            h3p = ps_x1.tile([P, BT], F32, tag="x1")
            nc.tensor.matmul(h3p[0:MLP[2], :],
                             lhsT=wall[:, W2_O:W2_O + MLP[2]], rhs=h2[:],
                             start=True, stop=True)
            h3 = hp.tile([MLP[2], BT], MMDT, tag="h3")
            nc.scalar.activation(h3[:], h3p[0:MLP[2], :],
                                 mybir.ActivationFunctionType.Relu,
                                 bias=aux[0:MLP[2], B3_O:B3_O + 1])

            # output layer into the fm accumulator
            nc.tensor.matmul(sp[SR_FM:SR_FM + 1, :],
                             lhsT=wall[0:MLP[2], WO_O:WO_O + 1],
                             rhs=h3[:], start=False, stop=True)

            # row 0 already holds the full presigmoid; sigmoid it
            ysb = yp.tile([1, BT], F32, tag="ysb")
            nc.scalar.activation(ysb[:], sp[SR_FM:SR_FM + 1, :],
                                 mybir.ActivationFunctionType.Sigmoid,
                                 bias=aux[0:1, SC_O:SC_O + 1])
            nc.sync.dma_start(out=y[t:t + 1, :], in_=ysb[:])

    nc.compile()
    return nc


_NC = None


def _get_nc():
    global _NC
    if _NC is None:
        _NC = _build_nc()
    return _NC


# ---------------------------------------------------------------- host prep
def _prep_shared(emb_table, bias_table, fm_bias, Wo, bo,
                 W0, b0, g0, be0, W1, b1, g1, be1, W2, b2, g2, be2):
    inv = np.float32(1.0 / np.sqrt(1.0 + BN_EPS))

    tab = np.empty([V, FW], np.float32)
    tab[:, :EMB] = emb_table
    tab[:, EMB] = bias_table[:, 0]

    # small-field subtable with 256B rows (zero padded), fields f2,f3,f5,f6
    tabs = np.zeros([VSM, GW], np.float32)
    for o, f in enumerate(SM_F):
        lo = int(OFFSETS[f])
        v = FIELD_DIMS[f]
        dst = int(SM_OFF[o])
        tabs[dst:dst + v, :FW] = tab[lo:lo + v]

    def fold(Wl, bl, gl, bel):
        s = (gl * inv).astype(np.float32)
        return (Wl * s[:, None]).astype(np.float32), (bl * s + bel).astype(np.float32)

    W0f, b0f = fold(W0, b0, g0, be0)
    W1f, b1f = fold(W1, b1, g1, be1)
    W2f, b2f = fold(W2, b2, g2, be2)

    # gathered-feature layout: row r (< 198) = field GATH_F[r//33], e = r%33
    # (e == 32 is the bias row); chunk0 = rows 0:128, chunk1 = rows 128:198
    # + dense at chunk-1 rows 70:86.
    w0t = np.zeros([2 * P, MLP[0]], np.float32)     # rows = chunk*128 + r
    fms = np.zeros([2 * P, SR_S + EMB], np.float32)  # col 0 = bias -> row 0
    fmq = np.zeros([2 * P, 1], np.float32)
    for g, f in enumerate(GATH_F):
        for e in range(FW):
            r = g * FW + e
            if e < EMB:
                w0t[r] = W0f[:, f * EMB + e]
                fms[r, SR_S + e] = 1.0
                fmq[r, 0] = -0.5
            else:
                fms[r, 0] = 1.0                 # bias row -> fm row 0
    for d in range(DENSE):
        w0t[P + DROW + d] = W0f[:, NF * EMB + d]

    wall = np.zeros([P, WALL_W], np.float32)
    wall[:, W0C0_O:W0C0_O + MLP[0]] = w0t[0:P]
    wall[:, W0C1_O:W0C1_O + MLP[0]] = w0t[P:2 * P]
    for k in range(2):
        wall[:, W1_O + k * MLP[1]:W1_O + (k + 1) * MLP[1]] = \
            W1f.T[k * P:(k + 1) * P]
    wall[:, W2_O:W2_O + MLP[2]] = W2f.T
    wall[0:MLP[2], WO_O] = Wo[0].astype(np.float32)
    wall[:, FMS0_O:FMS0_O + SR_S + EMB] = fms[0:P]
    wall[:, FMS1_O:FMS1_O + SR_S + EMB] = fms[P:2 * P]
    wall[:, FMQ0_O:FMQ0_O + 1] = fmq[0:P]
    wall[:, FMQ1_O:FMQ1_O + 1] = fmq[P:2 * P]
    wall[SR_S:SR_S + EMB, HV_O] = 0.5
    wall[0, ONES_O:ONES_O + P] = 1.0

    tiny = np.zeros([P, TINY_W], np.float32)
    for o, f in enumerate(OH_F):
        v = FIELD_DIMS[f]
        lo = int(OFFSETS[f])
        Tf = np.asarray(emb_table[lo:lo + v], np.float32)        # [v, 32]
        bf = np.asarray(bias_table[lo:lo + v, 0], np.float32)    # [v]
        Z = Tf @ W0f[:, f * EMB:(f + 1) * EMB].T                 # [v, 256]
        U = np.zeros([P, UW], np.float32)
        U[0:v, U_Z0:U_Z0 + MLP[0]] = Z
        U[0:v, U_S:U_S + EMB] = Tf
        U[0:v, U_BQ] = bf - 0.5 * (Tf * Tf).sum(axis=1)

        tiny[:, o * UW:(o + 1) * UW] = U
    tiny[:, TID_O:TID_O + P] = np.eye(P, dtype=np.float32)
    tiny = tiny.astype(ml_dtypes.bfloat16)

    auxa = np.zeros([P, AUX_W], np.float32)
    auxa[:, ID_O:ID_O + P] = np.eye(P, dtype=np.float32)
    for o in range(2):
        auxa[:, B1_O + o] = b0f[o * P:(o + 1) * P]
    auxa[:, B2_O] = b1f
    auxa[0:MLP[2], B3_O] = b2f
    auxa[0, SC_O] = np.float32(bo[0]) + np.float32(fm_bias[0])
    auxa[:, IOTA_O] = np.arange(P, dtype=np.float32)

    return dict(tab=tab, tabs=tabs, walld=wall, tinyd=tiny, auxd=auxa)


def _pack_small_idx(sm_loc):
    """sm_loc: [B_LOC, NSM] int64 local subtable rows ->
    [NT, P, NGATH*(NGI//16)] int16 in dma_gather's wrapped layout.
    Gather g of tile t covers subtiles 2g,2g+1: item j = (sl*NSM+fi)*128+p
    -> out chunk g*8 + sl*4 + fi, partition p; idx j lives at partition
    j%16, col j//16, replicated across the 8 groups of 16 partitions."""
    out = np.zeros([NT, NGATH, P, NGI // 16], np.int16)
    v = sm_loc.reshape(NT, NGATH, 2, P, NSM)            # [t, g, sl, p, fi]
    iv = v.transpose(0, 1, 2, 4, 3).reshape(NT, NGATH, NGI)  # j=(sl*4+fi)*128+p
    jj = np.arange(NGI)
    for t in range(NT):
        for g in range(NGATH):
            blk = np.zeros([16, NGI // 16], np.int16)
            blk[jj % 16, jj // 16] = iv[t, g].astype(np.int16)
            out[t, g] = np.tile(blk, (8, 1))
    return np.ascontiguousarray(out.transpose(0, 2, 1, 3).reshape(
        NT, P, NGATH * (NGI // 16)))


def _core_inputs(gl2, sm_loc, oh32, dense_inputs, c):
    lo = c * B_LOC
    g = gl2[lo:lo + B_LOC]                             # [2048, 2] global rows
    gidx = (g.reshape(NT, NS, P, NBIG)
            .transpose(0, 2, 1, 3)                     # [NT, 128, NS, NBIG]
            .reshape(NT, P, NS * NBIG))                # j = s*NBIG + f
    sidx = _pack_small_idx(sm_loc[lo:lo + B_LOC])
    o = oh32[lo:lo + B_LOC]                            # [2048, 4] local idx
    ohidx = (o.reshape(NT, NS, P, NOH)
             .transpose(0, 3, 1, 2)                    # [NT, NOH, NS, P]
             .reshape(NT, 1, NOH * BT))                # col = o*512 + s*128 + p
    dt_ = (dense_inputs[lo:lo + B_LOC]
           .reshape(NT, BT, DENSE)
           .transpose(0, 2, 1))                        # [NT, DENSE, BT]
    return (np.ascontiguousarray(gidx), sidx, np.ascontiguousarray(ohidx),
            np.ascontiguousarray(dt_))


def kernel(sparse_inputs, dense_inputs, emb_table, bias_table, fm_bias,
           Wo, bo, W0, b0, g0, be0, W1, b1, g1, be1, W2, b2, g2, be2):
    sparse_inputs = np.asarray(sparse_inputs)
    dense_inputs = np.asarray(dense_inputs, dtype=np.float32)
    args = [np.asarray(a, dtype=np.float32) for a in
            (emb_table, bias_table, fm_bias, Wo, bo,
             W0, b0, g0, be0, W1, b1, g1, be1, W2, b2, g2, be2)]
    shared = _prep_shared(*args)

    sp = sparse_inputs.astype(np.int64)
    glob = sp + OFFSETS[None, :]
    gl2 = glob[:, BIG_F].astype(np.int32)              # [B, 2] global rows
    sm_loc = sp[:, SM_F] + SM_OFF[None, :]             # [B, 4] subtable rows
    oh32 = sp[:, OH_F].astype(np.float32)              # [B, 4] local idx

    in_maps = []
    for c in range(N_CORES):
        gidx, sidx, ohidx, dt_ = _core_inputs(gl2, sm_loc, oh32,
                                              dense_inputs, c)
        in_maps.append(dict(shared, gidx=gidx, sidx=sidx, ohidx=ohidx,
                            denset=dt_))

    nc = _get_nc()
    res = run_bass_kernel_spmd(nc, in_maps, list(range(N_CORES)),
                               trace=bool(os.environ.get("BASS_TRACE")))
    kernel.last_results = res

    out = np.empty([B], np.float32)
    for c in range(N_CORES):
        out[c * B_LOC:(c + 1) * B_LOC] = res.results[c]["y"].reshape(-1)
    return out


# revision 34
# speedup vs baseline: 3.0435x; 3.0435x over previous
"""DeepFM (nn_DeepFM_77120432766994) Trainium2 Bass kernel.

Strategy: data-parallel over batch across 8 NeuronCores; per core 2048
batch rows in 4 tiles of 512.

Gather plan (the perf-critical part; SWDGE instructions cost ~1us fixed
each on the Pool engine, so instruction count dominates):
  - fields 2..9 (vocab 1008,1004,102,1000,500,50,112,107; 3883 rows
    total): ONE InstDMAGatherAnt per tile gathers all 8 fields x 512
    samples (4096 rows) from a concatenated subtable with 256B rows
    (64 f32: 32 emb + bias + zero pad) using int16 local indices.
  - fields 0,1 (vocab 1e6 / 1e5; indices exceed the gather's int16
    range): classic per-partition indirect DMA, 128 rows/instruction,
    8 instructions per tile, from a narrow [V, 33] f32 table.
  Total SWDGE instructions per tile: 9 (vs 40 for all-indirect).

Compute (as the prior all-indirect version):
  - staging tiles packed by DVE into G [128, 4, 330] f32 (sample-major,
    field f at cols 33f..33f+32 = emb dims + bias) so PE transposes see
    one source written exclusively by DVE (keeps every PE instruction
    at <= 1 new sync-wait).
  - 12 PE transposes -> X chunks [128feat, 512batch]; dense rows via
    host-transposed staging; K=90 partition slices exclude garbage.
  - FM: masked matmuls: S[e,b] (field sums; bias-sum in row 32),
    -0.5*sum(x^2) via squared X; 0.5*||s||^2 via ACT square + matmul.
  - MLP: matmul chains with fused bias+ReLU on the scalar engine; all
    accumulated into one [1,512] PSUM bank -> sigmoid -> y.
"""

import os
import sys

import numpy as np

for _p in ("/opt/trn_rl_repo", "/root/.axon_site/_ro/trn_rl_repo"):
    if os.path.isdir(_p) and _p not in sys.path:
        sys.path.insert(0, _p)

import concourse.bass as bass
import concourse.bacc as bacc
import concourse.mybir as mybir
import concourse.tile as tile
from concourse.bass_utils import run_bass_kernel_spmd

# ---------------------------------------------------------------- constants
FIELD_DIMS = [1000000, 100000, 1008, 1004, 102, 1000, 500, 50, 112, 107]
NF = 10
NSMALL = 8                    # fields 2..9 go through the bulk gather
NBIG = 2                      # fields 0,1 via indirect DMA
B = 16384
EMB = 32
DENSE = 16
MLP = [256, 128, 64]
BN_EPS = 1e-5
V = int(np.sum(FIELD_DIMS))
OFFSETS = np.concatenate([[0], np.cumsum(FIELD_DIMS)[:-1]]).astype(np.int64)
VSMALL = int(np.sum(FIELD_DIMS[2:]))          # 3883
SM_OFF = np.concatenate([[0], np.cumsum(FIELD_DIMS[2:])[:-1]]).astype(np.int64)

N_CORES = 8
B_LOC = B // N_CORES          # 2048
NT = 4                        # tiles per core
BT = 512                      # batch per tile
NS = 4                        # subtiles (of 128) per tile
P = 128

FW = 33                       # field width in feature layout (32 emb + bias)
GW = 64                       # gather row width (256B: FW used + zero pad)
D0 = NF * FW                  # 330 packed feature columns
NCH = 3                       # k-chunks: [0:128), [128:256), [256:346)
KC = [P, P, 112]              # contraction size per chunk (incl. dense rows)
DROW = 96                     # dense rows begin here within chunk 2 (32-aligned)

NSM_IDX = NS * P * NSMALL     # 4096 small-field rows gathered per tile
NGI = 1024                    # max num_idxs per gather (SWDGE ring capacity)
NGATH = NSM_IDX // NGI        # 4 gather instructions per tile

V1 = FIELD_DIMS[1]            # 100000
NW1 = 4                       # f1 windows of 32768 rows (int16-addressable)
W1SZ = 32768

F32 = mybir.dt.float32
F32R = mybir.dt.float32r
I32 = mybir.dt.int32
I16 = mybir.dt.int16

USE_F32R = True               # full-speed PE path; flip to False for exact fp32
MMDT = F32R if USE_F32R else F32

# wall (f32r weight blob) column offsets
W0_O = 0                      # 3 chunks x 256
W1_O = 768                    # 2 chunks x 128
W2_O = 1024                   # 64
WO_O = 1088                   # 1 (rows 0:64)
FM_O = 1089                   # 3 chunks x 34
HV_O = 1191                   # 1 (rows 0:32, value 0.5)
WALL_W = 1192
# aux (f32 blob) column offsets
ID_O = 0                      # identity 128
B1_O = 128                    # 2
B2_O = 130                    # 1
B3_O = 131                    # 1 (rows 0:64)
SC_O = 132                    # 1 (row 0: bo + fm_bias)
AUX_W = 133


# ---------------------------------------------------------------- device code
def _build_nc(reps=1):
    nc = bacc.Bacc("TRN2", target_bir_lowering=False, debug=False)

    tab = nc.dram_tensor("tab", [V, FW], F32, kind="ExternalInput").ap()
    tabs = nc.dram_tensor("tabs", [VSMALL, GW], F32, kind="ExternalInput").ap()
    tabs1 = nc.dram_tensor("tabs1", [V1, GW], F32, kind="ExternalInput").ap()
    gidx = nc.dram_tensor("gidx", [NT, P, NS], I32,
                          kind="ExternalInput").ap()
    f1idx = nc.dram_tensor("f1idx", [NT, P, NW1 * (BT // 16)], I16,
                           kind="ExternalInput").ap()
    f1msk = nc.dram_tensor("f1msk", [NT, P, NW1 * NS], F32,
                           kind="ExternalInput").ap()
    sidx = nc.dram_tensor("sidx", [NT, P, NGATH * (NGI // 16)], I16,
                          kind="ExternalInput").ap()
    denset = nc.dram_tensor("denset", [NT, DENSE, BT], F32,
                            kind="ExternalInput").ap()
    walld = nc.dram_tensor("walld", [P, WALL_W], MMDT, kind="ExternalInput").ap()
    auxd = nc.dram_tensor("auxd", [P, AUX_W], F32, kind="ExternalInput").ap()
    y = nc.dram_tensor("y", [NT, BT], F32, kind="ExternalOutput").ap()

    from contextlib import ExitStack
    with tile.TileContext(nc) as tc, ExitStack() as ctx:
        wp = ctx.enter_context(tc.tile_pool(name="weights", bufs=1))

        wall = wp.tile([P, WALL_W], MMDT, tag="wall")
        nc.sync.dma_start(out=wall[:], in_=walld[:])
        aux = wp.tile([P, AUX_W], F32, tag="aux")
        nc.sync.dma_start(out=aux[:], in_=auxd[:])
        dummy = wp.tile([1, 1], F32, tag="dummy")

        ident = aux[:, ID_O:ID_O + P]

        def w0(c, o):
            kc = KC[c]
            return wall[0:kc, W0_O + c * MLP[0] + o * P:W0_O + c * MLP[0] + (o + 1) * P]

        def w1(k):
            return wall[:, W1_O + k * MLP[1]:W1_O + (k + 1) * MLP[1]]

        def fmw_s(c):
            return wall[0:KC[c], FM_O + c * 34:FM_O + c * 34 + FW]

        def fmw_q(c):
            return wall[0:KC[c], FM_O + c * 34 + FW:FM_O + c * 34 + 34]

        ip = ctx.enter_context(tc.tile_pool(name="idx", bufs=2))
        sip = ctx.enter_context(tc.tile_pool(name="sidxp", bufs=2))
        gsp = ctx.enter_context(tc.tile_pool(name="gsmall", bufs=2))
        gfp = ctx.enter_context(tc.tile_pool(name="gf01", bufs=2))
        s1p = ctx.enter_context(tc.tile_pool(name="s1idx", bufs=2))
        m1p = ctx.enter_context(tc.tile_pool(name="m1", bufs=2))
        g1p = ctx.enter_context(tc.tile_pool(name="g1", bufs=2))
        g1tp = ctx.enter_context(tc.tile_pool(name="g1tmp", bufs=8))
        gp = ctx.enter_context(tc.tile_pool(name="gpack", bufs=2))
        dsp = ctx.enter_context(tc.tile_pool(name="dstage", bufs=2))
        xp = ctx.enter_context(tc.tile_pool(name="xchunks", bufs=6))
        xqp = ctx.enter_context(tc.tile_pool(name="xsq", bufs=2))
        s2p = ctx.enter_context(tc.tile_pool(name="s2", bufs=2))
        hp = ctx.enter_context(tc.tile_pool(name="acts", bufs=6))
        yp = ctx.enter_context(tc.tile_pool(name="yout", bufs=2))

        ps_x = ctx.enter_context(tc.tile_pool(name="ps_x", bufs=2, space="PSUM"))
        ps_s = ctx.enter_context(tc.tile_pool(name="ps_s", bufs=1, space="PSUM"))
        ps_fm = ctx.enter_context(tc.tile_pool(name="ps_fm", bufs=1, space="PSUM"))
        ps_h1 = ctx.enter_context(tc.tile_pool(name="ps_h1", bufs=2, space="PSUM"))
        ps_h2 = ctx.enter_context(tc.tile_pool(name="ps_h2", bufs=1, space="PSUM"))
        ps_h3 = ctx.enter_context(tc.tile_pool(name="ps_h3", bufs=1, space="PSUM"))

        # Absorber instructions: make PE/ACT observe the weight/aux DMA
        # semaphores via single-wait instructions before any real consumer.
        xps_a = ps_x.tile([P, BT], F32, tag="xps")
        nc.tensor.transpose(out=xps_a[:, 0:P], in_=ident, identity=ident)
        nc.tensor.transpose(out=xps_a[:, P:2 * P], in_=wall[:, 0:P].bitcast(F32),
                            identity=ident)
        nc.scalar.copy(dummy[:], aux[0:1, B1_O:B1_O + 1])

        import contextlib
        loop_cm = tc.For_i(0, reps, 1) if reps > 1 else contextlib.nullcontext()
        with loop_cm:
          for t in range(NT):
            # ---- small fields: 4 gathers of 1024 rows (ring cap), one per
            # subtile: gather g covers sample sub-block g's 8 fields.
            si = sip.tile([P, NGATH * (NGI // 16)], I16, tag="si")
            nc.sync.dma_start(out=si[:], in_=sidx[t])
            Gs = gsp.tile([P, (NSM_IDX // P) * GW], F32, tag="Gs")
            Gsv = Gs[:].rearrange("p (c e) -> p c e", c=NSM_IDX // P)
            W16 = NGI // 16
            for g in range(NGATH):
                nc.gpsimd.dma_gather(
                    out_ap=Gsv[:, g * (NGI // P):(g + 1) * (NGI // P), :],
                    in_ap=tabs,
                    idxs_ap=si[:, g * W16:(g + 1) * W16],
                    num_idxs=NGI,
                    num_idxs_reg=NGI,
                    elem_size=GW,
                    transpose=False,
                )

            # ---- field 0: per-partition indirect DMA (4 insts)
            idx = ip.tile([P, NS], I32, tag="idx")
            nc.sync.dma_start(out=idx[:], in_=gidx[t])
            Gf = gfp.tile([P, NS * FW], F32, tag="Gf")
            Gf3 = Gf[:].rearrange("p (s f) -> p s f", s=NS)
            for s in range(NS):
                nc.gpsimd.indirect_dma_start(
                    out=Gf3[:, s, :],
                    out_offset=None,
                    in_=tab,
                    in_offset=bass.IndirectOffsetOnAxis(
                        ap=idx[:, s:s + 1], axis=0),
                )

            # ---- field 1: 4 windowed dma_gathers (all 512 samples each,
            # dummy idx 0 when out-of-window) + mask-select merge on DVE.
            si1 = s1p.tile([P, NW1 * (BT // 16)], I16, tag="si1")
            nc.sync.dma_start(out=si1[:], in_=f1idx[t])
            mk1 = m1p.tile([P, NW1 * NS], F32, tag="mk1")
            nc.sync.dma_start(out=mk1[:], in_=f1msk[t])
            G1 = g1p.tile([P, NW1 * NS * GW], F32, tag="G1")
            G1v = G1[:].rearrange("p (w s e) -> p w s e", w=NW1, s=NS)
            W16B = BT // 16
            for w in range(NW1):
                lo = w * W1SZ
                hi = min(V1, lo + W1SZ)
                nc.gpsimd.dma_gather(
                    out_ap=G1v[:, w, :, :],
                    in_ap=tabs1[lo:hi],
                    idxs_ap=si1[:, w * W16B:(w + 1) * W16B],
                    num_idxs=BT,
                    num_idxs_reg=BT,
                    elem_size=GW,
                    transpose=False,
                )

            dstage = dsp.tile([DENSE, BT], F32, tag="dstage")
            nc.sync.dma_start(out=dstage[:], in_=denset[t])

            # ---- DVE packs -> G [128, NS, 330] (written exclusively by DVE)
            G = gp.tile([P, NS * D0], F32, tag="G")
            G3 = G[:].rearrange("p (s f) -> p s f", s=NS)
            # small rows: Gs[p, (s*8+fi), 0:33] -> G3[p, s, (2+fi)*33 ...]
            Gs4 = Gs[:].rearrange("p (s fi e) -> p s fi e", s=NS, fi=NSMALL)
            G4 = G[:].rearrange("p (s fi e) -> p s fi e", s=NS, fi=NF)
            nc.vector.tensor_copy(G4[:, :, NBIG:NF, 0:FW], Gs4[:, :, :, 0:FW])
            # f0 rows
            Gf4 = Gf[:].rearrange("p (s fi e) -> p s fi e", s=NS, fi=1)
            nc.vector.tensor_copy(G4[:, :, 0:1, :], Gf4[:])
            # f1 rows: sum_w G1[w] * mask[w]
            mk3 = mk1[:].rearrange("p (w s) -> p w s", w=NW1)
            g4f1 = G4[:, :, 1, 0:FW]
            nc.vector.tensor_tensor(
                out=g4f1, in0=G1v[:, 0, :, 0:FW],
                in1=mk3[:, 0, :, None].to_broadcast([P, NS, FW]),
                op=mybir.AluOpType.mult)
            for w in range(1, NW1):
                tmp = g1tp.tile([P, NS * FW], F32, tag="g1t")
                tmp3 = tmp[:].rearrange("p (s e) -> p s e", s=NS)
                nc.vector.tensor_tensor(
                    out=tmp3[:], in0=G1v[:, w, :, 0:FW],
                    in1=mk3[:, w, :, None].to_broadcast([P, NS, FW]),
                    op=mybir.AluOpType.mult)
                nc.vector.tensor_add(g4f1, g4f1, tmp3[:])

            # ---- transpose to feature-major chunks
            Xs = []
            for c in range(NCH):
                w = min(P, D0 - c * P)      # 128 / 128 / 74 gathered cols
                xps = ps_x.tile([P, BT], F32, tag="xps")
                for s in range(NS):
                    nc.tensor.transpose(
                        out=xps[0:w, s * P:(s + 1) * P],
                        in_=G3[:, s, c * P:c * P + w],
                        identity=ident,
                    )
                X = xp.tile([P, BT], MMDT, tag="X")
                if c == 2:
                    # zero the 64:96 band so the K=112 matmul reads no garbage
                    nc.vector.tensor_copy(
                        X[64:DROW, :],
                        wall[64:DROW, HV_O:HV_O + 1].to_broadcast([DROW - 64, BT]))
                nc.vector.tensor_copy(X[0:w, :], xps[0:w, :])
                if c == 2:
                    nc.vector.tensor_copy(X[DROW:DROW + DENSE, :], dstage[:])
                Xs.append(X)

            # ---- FM: S rows 0..31 = per-dim field sums, row 32 = bias sum
            sp = ps_s.tile([FW, BT], F32, tag="sp")
            for c in range(NCH):
                nc.tensor.matmul(sp[:], lhsT=fmw_s(c), rhs=Xs[c][0:KC[c], :],
                                 start=(c == 0), stop=(c == NCH - 1))

            fmp = ps_fm.tile([1, BT], F32, tag="fmp")
            for c in range(NCH):
                kc = KC[c]
                Xq = xqp.tile([P, BT], MMDT, tag="Xq")
                nc.vector.tensor_mul(Xq[0:kc, :], Xs[c][0:kc, :], Xs[c][0:kc, :])
                nc.tensor.matmul(fmp[:], lhsT=fmw_q(c), rhs=Xq[0:kc, :],
                                 start=(c == 0), stop=False)
            S2 = s2p.tile([EMB, BT], MMDT, tag="S2")
            nc.scalar.square(S2[:], sp[0:EMB, :])
            nc.tensor.matmul(fmp[:], lhsT=wall[0:EMB, HV_O:HV_O + 1], rhs=S2[:],
                             start=False, stop=False)

            # ---- MLP layer 0: h1[o, b], o in 2 chunks of 128
            h1s = []
            for o in range(2):
                h1p = ps_h1.tile([P, BT], F32, tag="h1p")
                for c in range(NCH):
                    nc.tensor.matmul(h1p[:], lhsT=w0(c, o), rhs=Xs[c][0:KC[c], :],
                                     start=(c == 0), stop=(c == NCH - 1))
                h1 = hp.tile([P, BT], MMDT, tag="h1")
                nc.scalar.activation(h1[:], h1p[:],
                                     mybir.ActivationFunctionType.Relu,
                                     bias=aux[:, B1_O + o:B1_O + o + 1])
                h1s.append(h1)

            # layer 1
            h2p = ps_h2.tile([P, BT], F32, tag="h2p")
            for k in range(2):
                nc.tensor.matmul(h2p[:], lhsT=w1(k), rhs=h1s[k][:],
                                 start=(k == 0), stop=(k == 1))
            h2 = hp.tile([P, BT], MMDT, tag="h2")
            nc.scalar.activation(h2[:], h2p[:],
                                 mybir.ActivationFunctionType.Relu,
                                 bias=aux[:, B2_O:B2_O + 1])

            # layer 2
            h3p = ps_h3.tile([MLP[2], BT], F32, tag="h3p")
            nc.tensor.matmul(h3p[:], lhsT=wall[:, W2_O:W2_O + MLP[2]], rhs=h2[:],
                             start=True, stop=True)
            h3 = hp.tile([MLP[2], BT], MMDT, tag="h3")
            nc.scalar.activation(h3[:], h3p[:],
                                 mybir.ActivationFunctionType.Relu,
                                 bias=aux[0:MLP[2], B3_O:B3_O + 1])

            # output layer into the FM accumulator
            nc.tensor.matmul(fmp[:], lhsT=wall[0:MLP[2], WO_O:WO_O + 1], rhs=h3[:],
                             start=False, stop=True)

            # presig = fmp + bias_sum row; y = sigmoid(presig + (bo + fm_bias))
            bsum = yp.tile([1, BT], F32, tag="bsum")
            nc.vector.tensor_copy(bsum[:], sp[EMB:FW, :])
            pres = yp.tile([1, BT], F32, tag="pres")
            nc.vector.tensor_add(pres[:], fmp[:], bsum[:])
            ysb = yp.tile([1, BT], F32, tag="ysb")
            nc.scalar.activation(ysb[:], pres[:],
                                 mybir.ActivationFunctionType.Sigmoid,
                                 bias=aux[0:1, SC_O:SC_O + 1])
            nc.sync.dma_start(out=y[t:t + 1, :], in_=ysb[:])

    nc.compile()
    return nc


_NC = None


def _get_nc():
    global _NC
    if _NC is None:
        _NC = _build_nc()
    return _NC


# ---------------------------------------------------------------- host prep
def _prep_shared(emb_table, bias_table, fm_bias, Wo, bo,
                 W0, b0, g0, be0, W1, b1, g1, be1, W2, b2, g2, be2):
    inv = np.float32(1.0 / np.sqrt(1.0 + BN_EPS))

    tab = np.empty([V, FW], np.float32)
    tab[:, :EMB] = emb_table
    tab[:, EMB] = bias_table[:, 0]

    # small-field subtable with 256B rows (zero padded)
    tabs = np.zeros([VSMALL, GW], np.float32)
    lo = int(OFFSETS[2])
    tabs[:, :FW] = tab[lo:lo + VSMALL]

    # f1 subtable with 256B rows for the windowed gathers
    tabs1 = np.zeros([V1, GW], np.float32)
    lo1 = int(OFFSETS[1])
    tabs1[:, :FW] = tab[lo1:lo1 + V1]

    def fold(Wl, bl, gl, bel):
        s = (gl * inv).astype(np.float32)
        return (Wl * s[:, None]).astype(np.float32), (bl * s + bel).astype(np.float32)

    W0f, b0f = fold(W0, b0, g0, be0)
    W1f, b1f = fold(W1, b1, g1, be1)
    W2f, b2f = fold(W2, b2, g2, be2)

    # feature permutation: model col 32f+e -> layout row 33f+e; dense -> 330+d
    w0t = np.zeros([NCH * P, MLP[0]], np.float32)
    for f in range(NF):
        w0t[f * FW:f * FW + EMB, :] = W0f[:, f * EMB:(f + 1) * EMB].T
    w0t[2 * P + DROW:2 * P + DROW + DENSE, :] = W0f[:, NF * EMB:].T

    fmw = np.zeros([NCH * P, 34], np.float32)
    for f in range(NF):
        for e in range(EMB):
            fmw[f * FW + e, e] = 1.0       # field-sum matrix
            fmw[f * FW + e, 33] = -0.5     # -0.5 * sum-of-squares mask
        fmw[f * FW + EMB, 32] = 1.0        # bias-sum mask

    wall = np.zeros([P, WALL_W], np.float32)
    for c in range(NCH):
        wall[:, W0_O + c * MLP[0]:W0_O + (c + 1) * MLP[0]] = w0t[c * P:(c + 1) * P]
    for k in range(2):
        wall[:, W1_O + k * MLP[1]:W1_O + (k + 1) * MLP[1]] = \
            W1f.T[k * P:(k + 1) * P]
    wall[:, W2_O:W2_O + MLP[2]] = W2f.T
    wall[0:MLP[2], WO_O] = Wo[0].astype(np.float32)
    for c in range(NCH):
        wall[:, FM_O + c * 34:FM_O + (c + 1) * 34] = fmw[c * P:(c + 1) * P]
    wall[0:EMB, HV_O] = 0.5

    auxa = np.zeros([P, AUX_W], np.float32)
    auxa[:, ID_O:ID_O + P] = np.eye(P, dtype=np.float32)
    for o in range(2):
        auxa[:, B1_O + o] = b0f[o * P:(o + 1) * P]
    auxa[:, B2_O] = b1f
    auxa[0:MLP[2], B3_O] = b2f
    auxa[0, SC_O] = np.float32(bo[0]) + np.float32(fm_bias[0])

    return dict(tab=tab, tabs=tabs, tabs1=tabs1, walld=wall, auxd=auxa)


def _pack_small_idx(sp_loc):
    """sp_loc: [B_LOC, NSMALL] int64 local small-table rows ->
    [NT, NGATH, 128, NGI//16] int16 in dma_gather's wrapped layout.
    Gather g of tile t covers subtile g: item j = fi*128+p lands at
    out[p, chunk g*8+fi] matching sample g*128+p field fi; idx j lives
    at partition j%16, col j//16, replicated across the 8 groups of 16
    partitions."""
    out = np.zeros([NT, NGATH, P, NGI // 16], np.int16)
    v = sp_loc.reshape(NT, NS, P, NSMALL)               # [t, g, p, fi]
    iv = v.transpose(0, 1, 3, 2).reshape(NT, NGATH, NGI)  # j = fi*128+p
    jj = np.arange(NGI)
    for t in range(NT):
        for g in range(NGATH):
            blk = np.zeros([16, NGI // 16], np.int16)
            blk[jj % 16, jj // 16] = iv[t, g].astype(np.int16)
            out[t, g] = np.tile(blk, (8, 1))
    # device tile holds gathers side by side: [t, p, g*(NGI//16)+w]
    return np.ascontiguousarray(out.transpose(0, 2, 1, 3).reshape(
        NT, P, NGATH * (NGI // 16)))


def _pack_f1(f1_loc):
    """f1_loc: [B_LOC] int64 local f1 rows -> (idx [NT,P,NW1*32] i16 in
    dma_gather's wrapped layout with dummy 0 when out-of-window,
    msk [NT,P,NW1*NS] f32 window-membership masks)."""
    idx_out = np.zeros([NT, NW1, P, BT // 16], np.int16)
    msk_out = np.zeros([NT, NW1, P, NS], np.float32)
    v = f1_loc.reshape(NT, BT)                          # j = s*128+p order
    jj = np.arange(BT)
    for t in range(NT):
        flat = v[t]
        for w in range(NW1):
            lo = w * W1SZ
            inw = (flat >= lo) & (flat < min(V1, lo + W1SZ))
            wi = np.where(inw, flat - lo, 0).astype(np.int16)
            blk = np.zeros([16, BT // 16], np.int16)
            blk[jj % 16, jj // 16] = wi
            idx_out[t, w] = np.tile(blk, (8, 1))
            msk_out[t, w] = inw.reshape(NS, P).T.astype(np.float32)
    return (np.ascontiguousarray(idx_out.transpose(0, 2, 1, 3).reshape(
                NT, P, NW1 * (BT // 16))),
            np.ascontiguousarray(msk_out.transpose(0, 2, 1, 3).reshape(
                NT, P, NW1 * NS)))


def _core_inputs(gl0, f1_loc, sp_loc, dense_inputs, c):
    lo = c * B_LOC
    g0 = gl0[lo:lo + B_LOC]                            # [2048] f0 global rows
    gidx = (g0.reshape(NT, NS, P)
            .transpose(0, 2, 1))                       # [NT, 128, NS]
    f1i, f1m = _pack_f1(f1_loc[lo:lo + B_LOC])
    sidx = _pack_small_idx(sp_loc[lo:lo + B_LOC])
    dt_ = (dense_inputs[lo:lo + B_LOC]
           .reshape(NT, BT, DENSE)
           .transpose(0, 2, 1))                        # [NT, DENSE, BT]
    return (np.ascontiguousarray(gidx), f1i, f1m, sidx,
            np.ascontiguousarray(dt_))


def kernel(sparse_inputs, dense_inputs, emb_table, bias_table, fm_bias,
           Wo, bo, W0, b0, g0, be0, W1, b1, g1, be1, W2, b2, g2, be2):
    sparse_inputs = np.asarray(sparse_inputs)
    dense_inputs = np.asarray(dense_inputs, dtype=np.float32)
    args = [np.asarray(a, dtype=np.float32) for a in
            (emb_table, bias_table, fm_bias, Wo, bo,
             W0, b0, g0, be0, W1, b1, g1, be1, W2, b2, g2, be2)]
    shared = _prep_shared(*args)

    sp = sparse_inputs.astype(np.int64)
    gl0 = (sp[:, 0] + OFFSETS[0]).astype(np.int32)                 # [B] f0
    f1_loc = sp[:, 1]                                              # [B] f1 local
    sp_loc = sp[:, NBIG:] + SM_OFF[None, :]                        # [B, 8] local

    in_maps = []
    for c in range(N_CORES):
        gidx, f1i, f1m, sidx, dt_ = _core_inputs(gl0, f1_loc, sp_loc,
                                                 dense_inputs, c)
        in_maps.append(dict(shared, gidx=gidx, f1idx=f1i, f1msk=f1m,
                            sidx=sidx, denset=dt_))

    nc = _get_nc()
    res = run_bass_kernel_spmd(nc, in_maps, list(range(N_CORES)),
                               trace=bool(os.environ.get("BASS_TRACE")))
    kernel.last_results = res

    out = np.empty([B], np.float32)
    for c in range(N_CORES):
        out[c * B_LOC:(c + 1) * B_LOC] = res.results[c]["y"].reshape(-1)
    return out

